# revision 64
# baseline (speedup 1.0000x reference)
"""Linear-attention kernel (out = (relu(Q)+eps) @ ((relu(K)+eps)^T V)) on 8 TRN2 cores.

Sharding: data-parallel over batch B=8 -> one batch per NeuronCore, no comm.
Per core: S=4096, D=256, DV=256, fp32 out.

Numerics: Q/K/V are cast to fp16 on the host (halves HBM->SBUF traffic; the
rounding point is identical to casting on-device). All matmul operands fp16,
PSUM accumulation fp32, output stored fp32.
"""

from contextlib import ExitStack

import numpy as np

import concourse.bacc as bacc
import concourse.bass as bass
import concourse.mybir as mybir
from concourse.bass_utils import run_bass_kernel_spmd
from concourse.masks import make_identity
from concourse.tile import TileContext

B, S, D, DV = 8, 4096, 256, 256
P = 128
NCH = S // P            # 32 chunks of 128 sequence rows
GRP = 8                 # chunks per DMA piece (512 KiB fp16)
NGRP = NCH // GRP       # 4
EPS = 1e-6
F32 = mybir.dt.float32
F16 = mybir.dt.float16
MAX = mybir.AluOpType.max
ADD = mybir.AluOpType.add
RELUF = mybir.ActivationFunctionType.Relu

_CACHE: dict = {}


def _build() -> bass.Bass:
    nc = bacc.Bacc("TRN2", target_bir_lowering=False)
    Kd = nc.declare_dram_parameter("K", [S, D], F16, isOutput=False)
    Vd = nc.declare_dram_parameter("V", [S, DV], F16, isOutput=False)
    Qd = nc.declare_dram_parameter("Q", [S, D], F16, isOutput=False)
    Od = nc.declare_dram_parameter("out", [S, DV], F32, isOutput=True)

    # seq row index s = p*NCH + n: partition-major so each partition's DMA
    # span is contiguous in DRAM.
    Kv = Kd[:, :].rearrange("(p n) d -> p n d", p=P)
    Vv = Vd[:, :].rearrange("(p n) d -> p n d", p=P)
    Qv = Qd[:, :].rearrange("(p n) d -> p n d", p=P)
    Ov = Od[:, :].rearrange("(p n) d -> p n d", p=P)

    with TileContext(nc) as tc, ExitStack() as ctx:
        consts = ctx.enter_context(tc.tile_pool(name="consts", bufs=1))
        big = ctx.enter_context(tc.tile_pool(name="big", bufs=1))
        pkv = ctx.enter_context(tc.tile_pool(name="pkv", bufs=1, space="PSUM"))
        pqt = ctx.enter_context(tc.tile_pool(name="pqt", bufs=3, space="PSUM"))
        pout = ctx.enter_context(tc.tile_pool(name="pout", bufs=3, space="PSUM"))

        ident = consts.tile([P, P], F16, name="ident")
        make_identity(nc, ident)
        epsb = consts.tile([P, 1], F32, name="epsb")
        nc.vector.memset(epsb, EPS)

        # Per-piece staging tiles (one DMA writer each, 512 KiB pieces).
        # Q splits its last piece in two: it bounds the final serial tail
        # (last transposes -> last phase-2 matmuls).
        KVP = [(0, 8), (8, 8), (16, 8), (24, 8)]
        QP = [(0, 8), (8, 8), (16, 8), (24, 4), (28, 4)]
        kts = [big.tile([P, w, D], F16, name=f"kt{i}") for i, (o, w) in enumerate(KVP)]
        vts = [big.tile([P, w, DV], F16, name=f"vt{i}") for i, (o, w) in enumerate(KVP)]
        qts = [big.tile([P, w, D], F16, name=f"qt{i}") for i, (o, w) in enumerate(QP)]
        qtT = big.tile([P, NCH, D], F16, name="qtT")   # (relu(Q)+eps)^T tiles
        ot = big.tile([P, NCH, DV], F32, name="ot")    # output staging
        kv = big.tile([P, 2, DV], F16, name="kv")      # KV = K_^T V, d-halves

        # Loads (HWDGE on Sync): K/V first at full bandwidth -- the critical
        # chain is K/V -> phase 1 -> KV -> phase 2. Q pieces trail; the
        # transposes and phase-2 matmuls they gate are cheap and pipeline
        # into the tail.
        def _ld(tile_, view, o, w):
            nc.sync.dma_start(out=tile_[:, :, :], in_=view[:, o:o + w, :])

        # K/V interleaved (K piece i lands before the V piece its matmuls
        # pair with), then Q pieces trail.
        for i, (o, w) in enumerate(KVP):
            _ld(kts[i], Kv, o, w)
            _ld(vts[i], Vv, o, w)
        for i, (o, w) in enumerate(QP):
            _ld(qts[i], Qv, o, w)

        # K relus on DVE in half-piece slices (the first matmuls gate on the
        # first slice, not a whole 512 KiB piece). Q needs no separate relu
        # pass: relu commutes with transpose, so it is fused into the
        # transpose copybacks below.
        for i, (o, w) in enumerate(KVP):
            hw_ = w // 2
            for half in range(2):
                sl = slice(half * hw_, (half + 1) * hw_)
                nc.vector.tensor_scalar(
                    out=kts[i][:, sl, :], in0=kts[i][:, sl, :],
                    scalar1=0.0, scalar2=EPS, op0=MAX, op1=ADD,
                )

        kvps = [pkv.tile([P, DV], F32, name=f"kvps{h}") for h in range(2)]

        # Warm the PE HAM clock-gate with dummy matmuls while the loads
        # stream in, so the real matmul stream starts closer to 2.4 GHz.
        ps_w = pout.tile([P, 2, DV], F32, name="ps_w", tag="ps_o")
        for i in range(12):
            nc.tensor.matmul(ps_w[:, 0, 0:P], ident[:, :], ident[:, :],
                             start=True, stop=True)

        def piece(pieces, n):
            for i, (o, w) in enumerate(pieces):
                if o <= n < o + w:
                    return i, n - o
            raise AssertionError(n)

        # Phase 1 back-to-back on the PE: KV[d, v] += K_[k, d] * V[k, v].
        for n in range(NCH):
            ki, kj = piece(KVP, n)
            for h in range(2):
                nc.tensor.matmul(
                    kvps[h][:, :],
                    kts[ki][:, kj, h * P:(h + 1) * P],
                    vts[ki][:, kj, :],
                    start=(n == 0), stop=(n == NCH - 1),
                )
        nc.vector.tensor_copy(kv[:, 0, :], kvps[0][:, :])
        nc.scalar.copy(kv[:, 1, :], kvps[1][:, :])

        # Tail: per Q piece, transpose its tiles on the PE (4 chunks x 2
        # halves batched into one PSUM bank + one wide relu-ing copyback),
        # then immediately run those chunks' phase-2 matmuls.
        alt = 0
        for qi, (o, w) in enumerate(QP):
            for q4 in range(w // 4):
                ps_t = pqt.tile([P, 8, P], F16, name="ps_t")
                for i2 in range(4):
                    j = q4 * 4 + i2
                    for h in range(2):
                        nc.tensor.transpose(
                            ps_t[:, i2 * 2 + h, :],
                            qts[qi][:, j, h * P:(h + 1) * P], ident,
                        )
                n0 = o + q4 * 4
                dst = qtT[:, n0:n0 + 4, :]
                # Copyback applies relu(x)+eps (post- == pre-transpose).
                if alt % 2 == 0:
                    nc.vector.tensor_scalar(
                        out=dst, in0=ps_t[:, :, :],
                        scalar1=0.0, scalar2=EPS, op0=MAX, op1=ADD,
                    )
                else:
                    nc.scalar.activation(dst, ps_t[:, :, :], RELUF,
                                         bias=epsb[:, :])
                alt += 1
            # Phase 2 for this piece's chunks, two chunks per PSUM bank.
            for n2 in range(w // 2):
                ps_o = pout.tile([P, 2, DV], F32, name="ps_o")
                for i2 in range(2):
                    n = o + n2 * 2 + i2
                    for h in range(2):
                        nc.tensor.matmul(
                            ps_o[:, i2, :],
                            qtT[:, n, h * P:(h + 1) * P],
                            kv[:, h, :],
                            start=(h == 0), stop=(h == 1),
                        )
                n0 = o + n2 * 2
                dst = ot[:, n0:n0 + 2, :]
                if n2 % 2 == 0:
                    nc.vector.tensor_copy(dst, ps_o[:, :, :])
                else:
                    nc.scalar.copy(dst, ps_o[:, :, :])
                # Alternate stores across both HWDGE rings (each FIFO-serial);
                # the final piece stores per 2 chunks to shorten the last
                # transfer on the critical tail.
                if o >= NCH - 4:
                    s = slice(n0, n0 + 2)
                    ring = nc.sync if (n0 // 2) % 2 == 0 else nc.scalar
                    ring.dma_start(out=Ov[:, s, :], in_=ot[:, s, :])
                elif (n0 + 2) % 4 == 0:
                    g4 = n0 // 4
                    s = slice(g4 * 4, (g4 + 1) * 4)
                    ring = nc.sync if g4 % 2 == 0 else nc.scalar
                    ring.dma_start(out=Ov[:, s, :], in_=ot[:, s, :])

    nc.compile()
    return nc


def _run(Q, K, V, trace=False, **trace_kwargs):
    if "nc" not in _CACHE:
        _CACHE["nc"] = _build()
    nc = _CACHE["nc"]
    Q = np.asarray(Q, dtype=np.float32).astype(np.float16)
    K = np.asarray(K, dtype=np.float32).astype(np.float16)
    V = np.asarray(V, dtype=np.float32).astype(np.float16)
    in_maps = [{"Q": Q[b], "K": K[b], "V": V[b]} for b in range(B)]
    res = run_bass_kernel_spmd(
        nc, in_maps, core_ids=list(range(B)), trace=trace, **trace_kwargs
    )
    out = np.stack([res.results[b]["out"] for b in range(B)], axis=0)
    return out, res


def kernel(Q, K, V):
    out, _ = _run(Q, K, V, trace=False)
    return out
